# revision 29
# baseline (speedup 1.0000x reference)
"""Trainium2 Bass kernel for WeightedCorrelationLayer (nn_CorrNetImpl).

Math (per batch b, G=1):
  x1 = temporal shift of x (frame t pairs with frame t-1; frame 0 with itself)
  x2 = spatially zero-padded x (pad=3)
  out[b, o=(dy,dx), t, h, w] = (1/C) * sum_c w[c,t,dy,dx] * x1[b,c,t,h,w]
                                       * x2[b,c,t,h+dy,w+dx]

Strategy:
  - Data-parallel over batch: core i handles batch i (B=8, 8 cores).
  - Per core, process t in pairs (t0,t1): SBUF tiles hold both frames on
    the 128 partitions as (f, c) -> partition f*64+c.
  - fp16 frames; even- and odd-aligned copies of the padded frame so
    every shifted view stays 4B-aligned for the DVE 2x perf mode.
  - Elementwise products P_o[k, hw] = x1[k, hw] * x2pad[k, (h+dy)*WPE
    + (w+dx)] split across VectorE (39 offsets, 2x_1p mode, dy-run
    batched) and GpSimd (10 offsets) so both engines land near the
    TensorE time; GpSimd is otherwise idle and runs concurrently.
  - Channel reduction on TensorE: ONE M=98 accumulation group per
    hw-chunk; PSUM row m = f*49 + o; each offset is one matmul with a
    host-precomputed [128, 98] lhsT block (weights/C baked in, col
    f*49+o nonzero on rows f*64..f*64+63), accumulating all 49 offsets.
  - ScalarE copies each chunk's PSUM to a staging tile (fp16), then 2
    contiguous DMAs per t-pair write DRAM (one per frame); host upcasts
    to f32 outside the measured kernel.
"""

import dataclasses

import numpy as np

import concourse.bacc as bacc
import concourse.mybir as mybir
import concourse.tile as tile
from concourse import bass_utils

B, C, T, H, W = 8, 64, 32, 56, 56
K = 7
PAD = (K - 1) // 2
NOFF = K * K
N_CORES = 8

F32 = mybir.dt.float32
FP16 = mybir.dt.float16

# dy-run split per dx parity, keyed by pool_n (total offsets GpSimd
# takes of 49). Pool gets the LOW dy rows so its (slower) products are
# consumed at the start of each dx column; per dx the value is
# (pool_runs, dve_runs) of (dy0, ndy).
FULL = ((0, 2), (2, 2), (4, 2), (6, 1))
SPLITS = {
    0: {0: ((), FULL), 1: ((), FULL)},
    4: {0: (((0, 1),), ((1, 2), (3, 2), (5, 2))),
        1: ((), FULL)},
    7: {0: (((0, 1),), ((1, 2), (3, 2), (5, 2))),
        1: (((0, 1),), ((1, 2), (3, 2), (5, 2)))},
    10: {0: (((0, 1),), ((1, 2), (3, 2), (5, 2))),
         1: (((0, 2),), ((2, 2), (4, 2), (6, 1)))},
    14: {0: (((0, 2),), ((2, 2), (4, 2), (6, 1))),
         1: (((0, 2),), ((2, 2), (4, 2), (6, 1)))}}


def _freeview(ap, dims, off):
    """Free-dim strided view of a flat [128, N] tile AP."""
    return dataclasses.replace(ap, ap=[ap.ap[0]] + dims, offset=ap.offset + off)


def build(C=C, T=T, H=H, W=W, K=K, chunk=448, mode="fp16", n_cores=N_CORES,
          p_bufs=8, pp_bufs=3, reps=1, pool_n=0, out_fp16=True,
          merge_runs=False, align2=False, borders=True, bench=None,
          pool_split=None):
    if pool_split is not None:  # back-compat alias
        pool_n = 10 if pool_split else 0

    def _merge(runs):
        """Coalesce contiguous (dy0, ndy) runs into maximal runs."""
        out = []
        for dy0, ndy in runs:
            if out and out[-1][0] + out[-1][1] == dy0:
                out[-1] = (out[-1][0], out[-1][1] + ndy)
            else:
                out.append((dy0, ndy))
        return tuple(out)
    assert mode == "fp16"
    PADL = (K - 1) // 2
    HW = H * W
    WPE = -(-(W + 2 * PADL) // 2) * 2  # even padded width for alignment
    HP = H + 2 * PADL
    NO = K * K
    NPAIR = T // 2
    M = 2 * NO  # psum rows: m = f*NO + o
    assert HW % chunk == 0 and chunk <= 512
    nchunk = HW // chunk

    nc = bacc.Bacc("TRN2", target_bir_lowering=False, debug=False,
                   num_devices=n_cores)
    x_d = nc.dram_tensor("x", [C, T, H, W], FP16, kind="ExternalInput")
    w_d = nc.dram_tensor("wblk", [NPAIR, 128, M * NO], FP16,
                         kind="ExternalInput")
    o_d = nc.dram_tensor("out", [NO, T, HW], FP16 if out_fp16 else F32,
                         kind="ExternalOutput")
    ODT = FP16 if out_fp16 else F32

    xap = x_d.ap()
    wap = w_d.ap()
    oap = o_d.ap()

    with tile.TileContext(nc) as tc:
        with (
            tc.tile_pool(name="x2", bufs=1) as x2pool,
            tc.tile_pool(name="x1", bufs=2) as x1pool,
            tc.tile_pool(name="wt", bufs=2) as wtpool,
            tc.tile_pool(name="prod", bufs=p_bufs) as ppool,
            tc.tile_pool(name="prodp", bufs=pp_bufs) as pppool,
            tc.tile_pool(name="ps", bufs=8, space="PSUM") as pspool,
            tc.tile_pool(name="ot", bufs=2) as otpool,
        ):
            NPL = HP * WPE  # padded plane size
            x2e = [x2pool.tile([128, NPL], FP16, tag=f"x2e_{i}",
                               name=f"x2e_{i}") for i in range(2)]
            x2o = [x2pool.tile([128, NPL], FP16, tag=f"x2o_{i}",
                               name=f"x2o_{i}") for i in range(2)] \
                if align2 else []
            for tl in x2e + x2o:
                nc.gpsimd.memset(tl[:, :], 0.0)

            for j in range(NPAIR * reps):
                j = j % NPAIR
                t0 = 2 * j
                t1 = t0 + 1

                x1t = x1pool.tile([128, HW], FP16, name="x1t")
                nc.sync.dma_start(
                    x1t[0:C, :],
                    xap[:, max(t0 - 1, 0)].rearrange("c h w -> c (h w)"))
                nc.sync.dma_start(
                    x1t[C:2 * C, :],
                    xap[:, t0].rearrange("c h w -> c (h w)"))
                x2et = x2e[j % 2]
                x2ot = x2o[j % 2] if align2 else x2et
                loads = ((x2et, PADL), (x2ot, PADL - 1)) if align2 \
                    else ((x2et, PADL),)
                for tl, c0 in loads:
                    v = tl[:, :].rearrange("p (h w) -> p h w", w=WPE)
                    nc.sync.dma_start(
                        v[0:C, PADL:PADL + H, c0:c0 + W], xap[:, t0])
                    nc.sync.dma_start(
                        v[C:2 * C, PADL:PADL + H, c0:c0 + W], xap[:, t1])

                wtt = wtpool.tile([128, M * NO], FP16, name="wtt")
                nc.sync.dma_start(wtt[:, :], wap[j])

                outt = otpool.tile([M, HW], ODT, name="outt")

                ps = {}
                n_emitted = 0
                for dx in range(K):
                    if dx % 2 == 0 or not align2:
                        src, xoff = (x2et, dx)
                    else:
                        src, xoff = x2ot, dx - 1
                    pool_runs, dve_runs = SPLITS[pool_n][dx % 2]
                    if merge_runs:
                        dve_runs = _merge(dve_runs)

                    for eng, pool_, runs in ((nc.gpsimd, pppool, pool_runs),
                                             (nc.vector, ppool, dve_runs)):
                        for dy0, ndy in runs:
                            ptw = (K if merge_runs and eng is nc.vector
                                   else 2) * HW
                            pt = pool_.tile([128, ptw], FP16, name="pt")
                            if eng is nc.gpsimd:
                                # Simple per-dy 2-D views (no 0-stride
                                # broadcast dim) for the Q7 software op.
                                for h in range(ndy):
                                    in1 = _freeview(
                                        src[:, :], [[WPE, H], [1, W]],
                                        (dy0 + h) * WPE + xoff)
                                    in0 = _freeview(
                                        x1t[:, :], [[W, H], [1, W]], 0)
                                    out = _freeview(
                                        pt[:, :], [[W, H], [1, W]], h * HW)
                                    eng.tensor_tensor(
                                        out, in0, in1,
                                        op=mybir.AluOpType.mult)
                            else:
                                if borders:
                                    w0 = max(0, PADL - dx)
                                    w1 = W - max(0, dx - PADL)
                                    h0r = max(0, PADL - (dy0 + ndy - 1))
                                    h1r = H - max(0, dy0 - PADL)
                                else:
                                    w0, w1, h0r, h1r = 0, W, 0, H
                                Wv, Hv = w1 - w0, h1r - h0r
                                in1 = _freeview(
                                    src[:, :],
                                    [[WPE, ndy], [WPE, Hv], [1, Wv]],
                                    (dy0 + h0r) * WPE + xoff + w0)
                                in0 = _freeview(
                                    x1t[:, :], [[0, ndy], [W, Hv], [1, Wv]],
                                    h0r * W + w0)
                                out = _freeview(
                                    pt[:, :], [[HW, ndy], [W, Hv], [1, Wv]],
                                    h0r * W + w0)
                                eng.tensor_tensor(out, in0, in1,
                                                  op=mybir.AluOpType.mult)
                                if borders:
                                    # zero pt borders so full-width matmul
                                    # reads accumulate exact zeros there
                                    zs = []
                                    if h0r:
                                        zs.append(([[HW, ndy],
                                                    [1, h0r * W]], 0))
                                    if h1r < H:
                                        zs.append(([[HW, ndy],
                                                    [1, (H - h1r) * W]],
                                                   h1r * W))
                                    if w0:
                                        zs.append(([[HW, ndy], [W, Hv],
                                                    [1, w0]], h0r * W))
                                    if w1 < W:
                                        zs.append(([[HW, ndy], [W, Hv],
                                                    [1, W - w1]],
                                                   h0r * W + w1))
                                    for dims, off in zs:
                                        nc.gpsimd.memset(
                                            _freeview(pt[:, :], dims, off),
                                            0.0)

                            for h in range(ndy):
                                if bench == "dve":
                                    break
                                dy = dy0 + h
                                o = dy * K + dx
                                first = (n_emitted == 0)
                                n_emitted += 1
                                last = (n_emitted == NO)
                                CR = chunk // W  # h-rows per chunk
                                if borders and not first:
                                    h0d = max(0, PADL - dy)
                                    h1d = H - max(0, dy - PADL)
                                else:
                                    # first matmul full: start-zeroes psum
                                    h0d, h1d = 0, H
                                for ch in range(nchunk):
                                    if first:
                                        ps[ch] = pspool.tile(
                                            [M, chunk], F32,
                                            name=f"ps_{j}_{ch}", tag="ps")
                                    if bench == "pe2":
                                        # double PE stream (garbage sums)
                                        nc.tensor.matmul(
                                            ps[ch][0:M, :],
                                            wtt[:, M * o:M * o + M],
                                            pt[:, h * HW + ch * chunk:
                                               h * HW + (ch + 1) * chunk],
                                            start=first, stop=False,
                                            skip_group_check=True)
                                    r0 = max(ch * CR, h0d)
                                    r1 = min((ch + 1) * CR, h1d)
                                    if r1 <= r0:
                                        continue
                                    nr = r1 - r0
                                    nc.tensor.matmul(
                                        _freeview(ps[ch][0:M, :],
                                                  [[W, nr], [1, W]],
                                                  (r0 - ch * CR) * W),
                                        wtt[:, M * o:M * o + M],
                                        _freeview(pt[:, :],
                                                  [[W, nr], [1, W]],
                                                  h * HW + r0 * W),
                                        start=(first and bench != "pe2"),
                                        stop=last,
                                        skip_group_check=(bench == "pe2"))
                                    if last:
                                        nc.scalar.copy(
                                            outt[0:M,
                                                 ch * chunk:(ch + 1) * chunk],
                                            ps[ch][0:M, :])

                if bench != "dve":
                    for f in range(2):
                        nc.sync.dma_start(
                            oap[:, t0 + f, :],
                            outt[f * NO:(f + 1) * NO, :])

    nc.compile()
    return nc


def make_wblk(filter_weight, C=C, T=T, K=K, mode="fp16"):
    """Host-side repack of (C, T, K, K) weights into per-pair lhsT blocks
    [NPAIR, 128, 98*49]. Offset o = dy*K+dx gets a [128, 98] block at
    cols 98*o: col f*49+o rows f*64..f*64+C-1 hold w[:, 2j+f, o]/C.
    Each block is the stationary operand of one M=98 matmul; PSUM row
    m = f*49 + o."""
    NO = K * K
    NPAIR = T // 2
    M = 2 * NO
    fw = np.asarray(filter_weight, np.float32).reshape(C, T, NO) / C
    wblk = np.zeros((NPAIR, 128, M * NO), np.float32)
    for o in range(NO):
        for f in range(2):
            wblk[:, f * C:(f + 1) * C, M * o + f * NO + o] = \
                fw[:, f::2, o].T
    return wblk.astype(np.float16)


_NC_CACHE = {}


def _get_nc(mode="fp16"):
    if mode not in _NC_CACHE:
        _NC_CACHE[mode] = build(mode=mode)
    return _NC_CACHE[mode]


def _get_exec(mode="fp16"):
    """Compile once per process: jitted shard_map executable over the 8
    cores, reused across kernel() calls (run_bass_kernel_spmd re-jits on
    every call)."""
    key = ("exec", mode)
    if key in _NC_CACHE:
        return _NC_CACHE[key]
    import jax
    from jax.sharding import Mesh, NamedSharding, PartitionSpec
    from jax.experimental.shard_map import shard_map
    import concourse.bass2jax as b2j

    nc = _get_nc(mode)
    b2j.install_neuronx_cc_hook()
    partition_name = (nc.partition_id_tensor.name
                      if nc.partition_id_tensor else None)
    in_names, out_names, out_avals, out_shapes = [], [], [], []
    for alloc in nc.m.functions[0].allocations:
        if not isinstance(alloc, mybir.MemoryLocationSet):
            continue
        name = alloc.memorylocations[0].name
        if alloc.kind == "ExternalInput":
            if name != partition_name:
                in_names.append(name)
        elif alloc.kind == "ExternalOutput":
            shape = tuple(alloc.tensor_shape)
            dtype = mybir.dt.np(alloc.dtype)
            out_names.append(name)
            out_shapes.append((shape, dtype))
            out_avals.append(jax.core.ShapedArray(shape, dtype))
    n_params = len(in_names)
    all_in = list(in_names) + list(out_names)
    if partition_name is not None:
        all_in.append(partition_name)

    def _body(*args):
        operands = list(args)
        if partition_name is not None:
            operands.append(b2j.partition_id_tensor())
        return tuple(b2j._bass_exec_p.bind(
            *operands, out_avals=tuple(out_avals), in_names=tuple(all_in),
            out_names=tuple(out_names), lowering_input_output_aliases=(),
            sim_require_finite=True, sim_require_nnan=True, nc=nc))

    devices = jax.devices()[:N_CORES]
    mesh = Mesh(np.asarray(devices), ("core",))
    specs = (PartitionSpec("core"),)
    fn = jax.jit(shard_map(_body, mesh=mesh,
                           in_specs=specs * (n_params + len(out_names)),
                           out_specs=specs * len(out_names), check_rep=False),
                 keep_unused=True)
    sharding = NamedSharding(mesh, PartitionSpec("core"))
    zeros = [jax.device_put(np.zeros((N_CORES * s[0], *s[1:]), d), sharding)
             for s, d in out_shapes]
    state = (fn, in_names, out_names, out_shapes, sharding, zeros, jax)
    _NC_CACHE[key] = state
    return state


def kernel(x, filter_weight, mode="fp16"):
    x = np.ascontiguousarray(np.asarray(x, np.float32).astype(np.float16))
    wblk = make_wblk(filter_weight, mode=mode)
    fn, in_names, out_names, out_shapes, sharding, zeros, jax = \
        _get_exec(mode)
    per_in = {"x": x.reshape(B * C, T, H, W),
              "wblk": np.concatenate([wblk] * N_CORES, axis=0)}
    args = [jax.device_put(per_in[n], sharding) for n in in_names] + zeros
    last_err = None
    for _attempt in range(3):
        try:
            outs = fn(*args)
            jax.block_until_ready(outs)
            break
        except Exception as e:  # transient device wedge: retry
            last_err = e
    else:
        raise last_err
    oi = out_names.index("out")
    out = np.asarray(outs[oi]).reshape(N_CORES, *out_shapes[oi][0])
    return out.reshape(B, NOFF, T, H, W).astype(np.float32)
